# revision 11
# baseline (speedup 1.0000x reference)
"""Trainium2 Bass kernel for nn_CauseEffectRepertoire (V3).

Computes, for each of 2 directions (cause/effect) and batch b:
    min over masks m of KL(full_b || 0.5*(softmax(MLP(state_b*bits_m)) +
                                          softmax(MLP(state_b*(1-bits_m)))))
with D=16, H=64, B=8, M=2^15-1=32767 masks, via an 8-core SPMD kernel that
shards the mask axis (4096 aligned masks per core, slot 0 = mask 0, ignored).

Design (what each chunk of 512 masks costs on-device):
  - First layer is a subset-sum over mask bits: for aligned 512-mask chunks,
    A[m] = Alo[m mod 512] + Ahi[m div 512].  Alo is a fixed fp16 SBUF tile
    per (pair, direction); Ahi + b1 folds into a per-chunk per-partition
    bias.  mm1 therefore disappears; relu_a = max(Alo + biasA, 0) and (via
    x_b = state - x_a) -relu_b = min(Alo - biasB, 0) are single
    tensor_scalar/activation ops reading fp16 SBUF.
  - Second layer, measured-cost mixed scheme: pairs {0,1} go through one
    fp8e4m3 DoubleRow matmul per side (dst partitions 0:64, the only dst
    the ISA allows for DoubleRow); pairs {2,3} go through plain fp16
    matmuls into partitions 64:128 (tile_position).  Both land in one
    (128, 1024) PSUM tile with partition layout P = 16*b + l.
  - One (128, 1024) exp on ACT converts logits to E = exp(L2+b2-SHIFT) in
    fp16, DMA'd to DRAM.  Host computes Z/u/s, ranks, and re-evaluates
    candidates within DELTA of the device max in float64 (s = const -
    ln2*KL exactly; measured fp8/fp16 ranking noise is ~0.004 << DELTA).
  - Engine budget per chunk-dir: DVE 6 relus (fast 2x SBUF mode), ACT 2
    relus + exp, Pool idle (its tensor_scalar ucode measures ~7.6us per op
    on hw and stalls concurrent DVE ops - unusable).
"""

import os
import sys
from contextlib import ExitStack

import numpy as np
import ml_dtypes

sys.path.insert(0, "/opt/trn_rl_repo")

D, H, B = 16, 64, 8
M = 2**15  # 32768 slots; slot 0 = mask 0 (excluded from ranking)
NCORES = 8
MC = M // NCORES  # 4096 masks per core
CHUNK = 1024
NCHUNK = MC // CHUNK  # 4
NPAIR = 4  # batch pairs; pair p = batches (2p, 2p+1)
B2SHIFT = 2.0  # uniform logit shift; s-ranking invariant, keeps E in range
DR_PAIRS = (0, 1)  # pairs routed through the fp8 DoubleRow matmul

# engine per (pair, side): 'dve' | 'act'.  side 0 = a, side 1 = b.
# b-sides must emit -relu_b (sub+min), which ACT cannot, so they sit on DVE;
# the -1 sign is baked into the matmul stationaries.
RELU_ENG = {
    (0, 0): "dve", (0, 1): "dve",
    (1, 0): "act", (1, 1): "dve",
    (2, 0): "dve", (2, 1): "dve",
    (3, 0): "act", (3, 1): "dve",
}

_f32 = np.float32
_f16 = np.float16
_f8 = ml_dtypes.float8_e4m3


def _relu_sign(p, side):
    """Sign the assigned engine writes: a-side +relu_a; b-side on DVE is
    -relu_b (min(Alo-biasB, 0)); b-side on ACT would be +relu_b."""
    if side == 0:
        return 1.0
    return 1.0 if RELU_ENG[(p, side)] == "act" else -1.0


def _host_prep(inputs):
    """Build all device input arrays (float64 math, fp16/fp8 outputs)."""
    state = np.asarray(inputs["state"], dtype=np.float64)  # (B, D)
    dirs = []
    for pre in ("cause", "effect"):
        dirs.append(
            tuple(
                np.asarray(inputs[f"{pre}_{k}"], dtype=np.float64)
                for k in ("w1", "b1", "w2", "b2")
            )
        )

    bitslo = ((np.arange(CHUNK)[:, None] >> np.arange(10)[None, :]) & 1).astype(
        np.float64
    )  # (1024, 10): low-10-bit patterns

    # V[b, j, d] = state[b, d] * w1[j, d], per direction
    Vs = [state[:, None, :] * dirs[d_][0][None, :, :] for d_ in range(2)]

    # alo: (128, 8*512) fp16; block q = dir*4 + p holds Alo[pi=(64*b2+j), r]
    alo = np.zeros((128, 8 * CHUNK))
    for d_ in range(2):
        for p in range(NPAIR):
            q = d_ * NPAIR + p
            for b2 in range(2):
                b = 2 * p + b2
                blk = bitslo @ Vs[d_][b, :, :10].T  # (1024, 64)
                alo[64 * b2 : 64 * b2 + 64, q * CHUNK : (q + 1) * CHUNK] = blk.T

    # w2s8: fp8 DoubleRow stationaries for pairs {0,1}: (128, 4*128);
    # block s_idx = dir*2 + side is [pi, i*64 + t], t = 32*i + 16*b2 + l
    w2s8 = np.zeros((128, 4 * 128))
    for d_ in range(2):
        w2 = dirs[d_][2]  # (16, 64)
        for side in range(2):
            blk = np.zeros((128, 2, 64))
            for i, p in enumerate(DR_PAIRS):
                sgn = _relu_sign(p, side)
                for b2 in range(2):
                    t0 = 32 * i + 16 * b2
                    blk[64 * b2 : 64 * b2 + 64, i, t0 : t0 + 16] = sgn * w2.T
            s_idx = d_ * 2 + side
            w2s8[:, s_idx * 128 : (s_idx + 1) * 128] = blk.reshape(128, 128)

    # w2m: fp16 plain stationaries for pairs {2,3}: (128, 4*32); block
    # s_idx = dir*2 + side maps (b2, j) rows -> 16*b2 + l cols
    w2m = np.zeros((128, 4 * 32))
    for d_ in range(2):
        w2 = dirs[d_][2]
        for side in range(2):
            blk = np.zeros((128, 32))
            for b2 in range(2):
                # sign may differ per pair; plain stationary is shared by
                # pairs 2 and 3, so their signs must match
                sgns = {_relu_sign(p, side) for p in (2, 3)}
                assert len(sgns) == 1, "pairs 2,3 must share side sign"
                blk[64 * b2 : 64 * b2 + 64, 16 * b2 : 16 * b2 + 16] = (
                    sgns.pop() * w2.T
                )
            s_idx = d_ * 2 + side
            w2m[:, s_idx * 32 : (s_idx + 1) * 32] = blk

    # b2t: (128, 2); output partition P = 16*b + l -> l = P % 16
    b2t = np.zeros((128, 2))
    for d_ in range(2):
        b2t[:, d_] = np.tile(dirs[d_][3], B) - B2SHIFT

    # per-core biases: (128, 64) fp32; col = q*8 + n
    in_maps = []
    for c in range(NCORES):
        biasA = np.zeros((128, 8 * NCHUNK))
        biasB = np.zeros((128, 8 * NCHUNK))
        for d_ in range(2):
            w1, b1, w2, b2 = dirs[d_]
            V = Vs[d_]
            C = V.sum(-1)  # (B, 64): full-state preact (all 16 d's)
            for p in range(NPAIR):
                q = d_ * NPAIR + p
                for n in range(NCHUNK):
                    base = (c * NCHUNK + n) * CHUNK
                    hb = (base >> np.arange(10, 16)) & 1  # bits 10..15
                    col = q * NCHUNK + n
                    for b2i in range(2):
                        b = 2 * p + b2i
                        ahi = V[b, :, 10:16] @ hb.astype(np.float64)  # (64,)
                        sl = slice(64 * b2i, 64 * b2i + 64)
                        biasA[sl, col] = ahi + b1
                        biasB[sl, col] = C[b] + b1 - ahi
        in_maps.append(
            {
                "alo": alo.astype(_f16),
                "w2s8": w2s8.astype(_f8),
                "w2m": w2m.astype(_f16),
                "b2t": b2t.astype(_f32),
                "biasA": biasA.astype(_f32),
                "biasB": biasB.astype(_f32),
            }
        )

    return in_maps, None, dirs, state


def _patch_act_tables():
    """Force every activation to resolve to natural_log_exp_and_others
    (contains Ln, Exp, Relu, Copy, Identity) so the kernel pays exactly one
    ACT table load."""
    import concourse.bacc as bacc_mod
    from concourse import hw_specs

    if getattr(bacc_mod, "_act_tables_patched", False):
        return
    orig = hw_specs.get_activation_tables

    def only_nle(arch):
        t = dict(orig(arch))
        if "natural_log_exp_and_others" in t:
            t = {
                k: (v if k == "natural_log_exp_and_others" else set())
                for k, v in t.items()
            }
        return t

    bacc_mod.get_activation_tables = only_nle
    bacc_mod._act_tables_patched = True


_NC_CACHE = {}


def build_nc(repeats=1):
    """Build and compile the 8-core SPMD Bass program (cached)."""
    if repeats in _NC_CACHE:
        return _NC_CACHE[repeats]

    import concourse.bacc as bacc
    import concourse.tile as tile
    from concourse import mybir

    _patch_act_tables()

    AF = mybir.ActivationFunctionType
    OP = mybir.AluOpType
    PM = mybir.MatmulPerfMode
    f32 = mybir.dt.float32
    f16 = mybir.dt.float16
    f8 = mybir.dt.float8e4

    nc = bacc.Bacc(
        "TRN2", target_bir_lowering=False, debug=False, num_devices=NCORES
    )

    ins = {}
    dts = {
        "alo": f16,
        "w2s8": f8,
        "w2m": f16,
        "b2t": f32,
        "biasA": f32,
        "biasB": f32,
    }
    for name, shape in (
        ("alo", (128, 8 * CHUNK)),
        ("w2s8", (128, 4 * 128)),
        ("w2m", (128, 4 * 32)),
        ("b2t", (128, 2)),
        ("biasA", (128, 8 * NCHUNK)),
        ("biasB", (128, 8 * NCHUNK)),
    ):
        ins[name] = nc.dram_tensor(name, shape, dts[name], kind="ExternalInput").ap()
    out_d = nc.dram_tensor(
        "edump", (128, 2 * NCHUNK * 2 * CHUNK), f16, kind="ExternalOutput"
    ).ap()

    with tile.TileContext(nc) as tc, ExitStack() as ctx:
        cpool = ctx.enter_context(tc.tile_pool(name="consts", bufs=1))
        xpool = ctx.enter_context(tc.tile_pool(name="xrelu", bufs=2))
        epool = ctx.enter_context(tc.tile_pool(name="eout", bufs=3))
        pp_mid = ctx.enter_context(tc.tile_pool(name="pmid", bufs=2, space="PSUM"))

        ct = {}
        for name in ins:
            shp = list(ins[name].shape)
            t = cpool.tile(shp, dts[name], tag=name, name=f"c_{name}")
            if name == "alo":
                # split the 8KB/partition load so the first chunk's relus
                # only wait on their own pair blocks
                for q in range(8):
                    sl = slice(q * CHUNK, (q + 1) * CHUNK)
                    nc.sync.dma_start(t[:, sl], ins[name][:, sl])
            else:
                nc.sync.dma_start(t[:], ins[name][:])
            ct[name] = t

        rep_ctx = tc.For_i(0, repeats, 1) if repeats > 1 else None
        if rep_ctx is not None:
            rep_ctx.__enter__()

        for d_ in range(2):
            for n in range(NCHUNK):
                k = d_ * NCHUNK + n
                # X8[side]: DoubleRow moving for pairs {0,1}: [128, (i, r)]
                X8 = [
                    xpool.tile([128, 2 * CHUNK], f8, tag=f"x8{s}",
                               name=f"x8{s}_{k}")
                    for s in range(2)
                ]
                # Xp[p-2][side]: plain fp16 moving for pairs {2,3}
                Xp = [
                    [
                        xpool.tile([128, CHUNK], f16, tag=f"xp{p}{s}",
                                   name=f"xp{p}{s}_{k}")
                        for s in range(2)
                    ]
                    for p in (2, 3)
                ]
                for p in range(NPAIR):
                    q = d_ * NPAIR + p
                    alo_blk = ct["alo"][:, q * CHUNK : (q + 1) * CHUNK]
                    col = q * NCHUNK + n
                    for side in range(2):
                        bias = (
                            ct["biasA"][:, col : col + 1]
                            if side == 0
                            else ct["biasB"][:, col : col + 1]
                        )
                        if p in DR_PAIRS:
                            i = DR_PAIRS.index(p)
                            dst = X8[side][:, i * CHUNK : (i + 1) * CHUNK]
                        else:
                            dst = Xp[p - 2][side][:]
                        eng = RELU_ENG[(p, side)]
                        if side == 0:
                            # +relu_a = max(Alo + biasA, 0)
                            if eng == "act":
                                nc.scalar.activation(
                                    dst, alo_blk, AF.Relu, bias=bias
                                )
                            else:
                                nc.vector.tensor_scalar(
                                    dst, alo_blk, bias, 0.0, OP.add, OP.max
                                )
                        else:
                            if eng == "act":
                                # +relu_b = relu(-Alo + biasB)
                                nc.scalar.activation(
                                    dst, alo_blk, AF.Relu, bias=bias,
                                    scale=-1.0
                                )
                            else:
                                # -relu_b = min(Alo - biasB, 0)
                                nc.vector.tensor_scalar(
                                    dst, alo_blk, bias, 0.0, OP.subtract,
                                    OP.min
                                )

                # second layer into one (128, 1024) PSUM tile, P = 16b + l:
                # DoubleRow (pairs 0,1) -> partitions 0:64; plain fp16
                # (pairs 2,3) -> partitions 64:128 via tile_position
                # group matmuls by PE mode (DoubleRow first, then plain) —
                # mode switches flush the PE pipeline (~190 ns each).  Each
                # matmul is split into 512-col halves: a matmul output may
                # not span a 2KB PSUM bank (s3d3_mm_num_elements).
                L2 = pp_mid.tile([128, 2 * CHUNK], f32, tag="L2", name=f"L2_{k}")
                HB = CHUNK // 512
                for side in range(2):
                    s_idx = d_ * 2 + side
                    w_blk = ct["w2s8"][
                        :, s_idx * 128 : (s_idx + 1) * 128
                    ].rearrange("p (i t) -> p i t", i=2)
                    x3 = X8[side][:].rearrange("p (i n) -> p i n", i=2)
                    for h in range(HB):
                        nc.tensor.matmul(
                            L2[0:64, side * CHUNK + h * 512 :
                               side * CHUNK + (h + 1) * 512],
                            w_blk,
                            x3[:, :, h * 512 : (h + 1) * 512],
                            start=True,
                            stop=True,
                            perf_mode=PM.DoubleRow,
                            tile_position=(0, 0),
                        )
                for side in range(2):
                    s_idx = d_ * 2 + side
                    for p in (2, 3):
                        for h in range(HB):
                            nc.tensor.matmul(
                                L2[32 * p : 32 * p + 32,
                                   side * CHUNK + h * 512 :
                                   side * CHUNK + (h + 1) * 512],
                                ct["w2m"][:, s_idx * 32 : (s_idx + 1) * 32],
                                Xp[p - 2][side][:, h * 512 : (h + 1) * 512],
                                start=True,
                                stop=True,
                                tile_position=(0, 32 * p),
                            )

                # E = exp(L2 + b2 - SHIFT), fp16, straight to DRAM
                E = epool.tile([128, 2 * CHUNK], f16, tag="E", name=f"E_{k}")
                nc.scalar.activation(
                    E[:], L2[:], AF.Exp, bias=ct["b2t"][:, d_ : d_ + 1]
                )
                nc.sync.dma_start(
                    out_d[:, k * 2 * CHUNK : (k + 1) * 2 * CHUNK], E[:]
                )

        if rep_ctx is not None:
            rep_ctx.__exit__(None, None, None)

    nc.compile()
    _NC_CACHE[repeats] = nc
    return nc


DELTA = 0.05  # nats: candidate margin below the device max (>> fp8 noise)


def kernel(**inputs):
    from concourse.bass_utils import run_bass_kernel_spmd

    in_maps, _, dirs, state = _host_prep(inputs)
    nc = build_nc()
    res = run_bass_kernel_spmd(nc, in_maps, list(range(NCORES)))
    ed = np.stack([r["edump"].astype(np.float32) for r in res.results])

    # ed: (core, 128, 16*1024); block k=dir*8+n: [side a 512 | side b 512];
    # partition P = 16*b + l; slot value m = (c*8+n)*512 + r
    ed = ed.reshape(NCORES, B, D, 2, NCHUNK, 2, CHUNK)  # (c, b, l, dir, n, s, r)
    E = ed.transpose(3, 5, 0, 4, 6, 1, 2).reshape(2, 2, M, B, D)
    Ea, Eb = E[:, 0], E[:, 1]  # (2, M, B, D)
    Za = Ea.sum(-1, keepdims=True)
    Zb = Eb.sum(-1, keepdims=True)
    u = Ea * Zb + Eb * Za
    lnu = np.log(np.maximum(u, 1e-30))
    s_all = np.empty((2, M, B), np.float32)
    for d_ in range(2):
        w1, b1, w2, b2 = dirs[d_]

        def mlp(x):
            h = np.maximum(x @ w1.T + b1, 0.0)
            lg = h @ w2.T + b2
            lg = lg - lg.max(axis=-1, keepdims=True)
            e = np.exp(lg)
            return e / e.sum(axis=-1, keepdims=True)

        full = mlp(state)  # (B, D)
        s_all[d_] = np.einsum("mbd,bd->mb", lnu[d_], full) - np.log(
            u[d_].sum(axis=-1)
        )
    s_all[:, 0, :] = -np.inf  # slot 0 = mask 0: not a valid partition

    # exact float64 re-evaluation of near-max candidates
    out = np.zeros((2, B))
    st = state  # (B, D) float64
    for d_ in range(2):
        sm = s_all[d_]  # (M, B)
        thr = sm.max(axis=0) - DELTA
        cand = np.where((sm >= thr[None, :]).any(axis=1))[0]
        bsel = ((cand[:, None] >> np.arange(D)[None, :]) & 1).astype(
            np.float64
        )  # (K, D)
        w1, b1, w2, b2 = dirs[d_]

        def mlp(x):
            h = np.maximum(x @ w1.T + b1, 0.0)
            lg = h @ w2.T + b2
            lg = lg - lg.max(axis=-1, keepdims=True)
            e = np.exp(lg)
            return e / e.sum(axis=-1, keepdims=True)

        full = mlp(st)  # (B, D)
        sa = mlp(st[None, :, :] * bsel[:, None, :])  # (K, B, D)
        sb = mlp(st[None, :, :] * (1.0 - bsel)[:, None, :])
        mix = 0.5 * (sa + sb)
        kl = (full[None] * (np.log2(full[None]) - np.log2(mix))).sum(-1)  # (K, B)
        out[d_] = kl.min(axis=0)
    return out.astype(np.float32)


if __name__ == "__main__":
    import jax

    import reference

    cpu = jax.devices("cpu")[0]
    with jax.default_device(cpu):
        inp = reference.setup_inputs()
        inp = {k: np.asarray(jax.device_put(v, cpu)) for k, v in inp.items()}
    out = kernel(**inp)
    print(out)


# revision 17
# speedup vs baseline: 1.1216x; 1.1216x over previous
"""Trainium2 Bass kernel for nn_CauseEffectRepertoire (V3).

Computes, for each of 2 directions (cause/effect) and batch b:
    min over masks m of KL(full_b || 0.5*(softmax(MLP(state_b*bits_m)) +
                                          softmax(MLP(state_b*(1-bits_m)))))
with D=16, H=64, B=8, M=2^15-1=32767 masks, via an 8-core SPMD kernel that
shards the mask axis (4096 aligned masks per core, slot 0 = mask 0, ignored).

Design (what each chunk of 512 masks costs on-device):
  - First layer is a subset-sum over mask bits: for aligned 512-mask chunks,
    A[m] = Alo[m mod 512] + Ahi[m div 512].  Alo is a fixed fp16 SBUF tile
    per (pair, direction); Ahi + b1 folds into a per-chunk per-partition
    bias.  mm1 therefore disappears; relu_a = max(Alo + biasA, 0) and (via
    x_b = state - x_a) -relu_b = min(Alo - biasB, 0) are single
    tensor_scalar/activation ops reading fp16 SBUF.
  - Second layer, measured-cost mixed scheme: pairs {0,1} go through one
    fp8e4m3 DoubleRow matmul per side (dst partitions 0:64, the only dst
    the ISA allows for DoubleRow); pairs {2,3} go through plain fp16
    matmuls into partitions 64:128 (tile_position).  Both land in one
    (128, 1024) PSUM tile with partition layout P = 16*b + l.
  - One (128, 1024) exp on ACT converts logits to E = exp(L2+b2-SHIFT) in
    fp16, DMA'd to DRAM.  Host computes Z/u/s, ranks, and re-evaluates
    candidates within DELTA of the device max in float64 (s = const -
    ln2*KL exactly; measured fp8/fp16 ranking noise is ~0.004 << DELTA).
  - Engine budget per chunk-dir: DVE 6 relus (fast 2x SBUF mode), ACT 2
    relus + exp, Pool idle (its tensor_scalar ucode measures ~7.6us per op
    on hw and stalls concurrent DVE ops - unusable).
"""

import os
import sys
from contextlib import ExitStack

import numpy as np
import ml_dtypes

sys.path.insert(0, "/opt/trn_rl_repo")

D, H, B = 16, 64, 8
M = 2**15  # 32768 slots; slot 0 = mask 0 (excluded from ranking)
NCORES = 8
MC = M // NCORES  # 4096 masks per core
CHUNK = 1024
NCHUNK = MC // CHUNK  # 4
NPAIR = 4  # batch pairs; pair p = batches (2p, 2p+1)
B2SHIFT = 2.0  # uniform logit shift; s-ranking invariant, keeps E in range
DR_PAIRS = (0, 1)  # pairs routed through the fp8 DoubleRow matmul

# engine per (pair, side): 'dve' | 'act'.  side 0 = a, side 1 = b.
# b-sides must emit -relu_b (sub+min), which ACT cannot, so they sit on DVE;
# the -1 sign is baked into the matmul stationaries.  All relu outputs are
# fp8 (measured: DVE fp8-out 1024-col ops are FASTER than fp16-out, and the
# fp8 moving operand costs the PE nothing extra); DVE takes 7, ACT 1 + exp.
RELU_ENG = {
    (0, 0): "dve", (0, 1): "dve",
    (1, 0): "dve", (1, 1): "dve",
    (2, 0): "dve", (2, 1): "dve",
    (3, 0): "act", (3, 1): "dve",
}

_f32 = np.float32
_f16 = np.float16
_f8 = ml_dtypes.float8_e4m3


def _relu_sign(p, side):
    """Sign the assigned engine writes: a-side +relu_a; b-side on DVE is
    -relu_b (min(Alo-biasB, 0)); b-side on ACT would be +relu_b."""
    if side == 0:
        return 1.0
    return 1.0 if RELU_ENG[(p, side)] == "act" else -1.0


def _host_prep(inputs):
    """Build all device input arrays (float64 math, fp16/fp8 outputs)."""
    state = np.asarray(inputs["state"], dtype=np.float64)  # (B, D)
    dirs = []
    for pre in ("cause", "effect"):
        dirs.append(
            tuple(
                np.asarray(inputs[f"{pre}_{k}"], dtype=np.float64)
                for k in ("w1", "b1", "w2", "b2")
            )
        )

    bitslo = ((np.arange(CHUNK)[:, None] >> np.arange(10)[None, :]) & 1).astype(
        np.float64
    )  # (1024, 10): low-10-bit patterns

    # V[b, j, d] = state[b, d] * w1[j, d], per direction
    Vs = [state[:, None, :] * dirs[d_][0][None, :, :] for d_ in range(2)]

    # alo: (128, 8*512) fp16; block q = dir*4 + p holds Alo[pi=(64*b2+j), r]
    alo = np.zeros((128, 8 * CHUNK))
    for d_ in range(2):
        for p in range(NPAIR):
            q = d_ * NPAIR + p
            for b2 in range(2):
                b = 2 * p + b2
                blk = bitslo @ Vs[d_][b, :, :10].T  # (1024, 64)
                alo[64 * b2 : 64 * b2 + 64, q * CHUNK : (q + 1) * CHUNK] = blk.T

    # w2s8: fp8 DoubleRow stationaries for pairs {0,1}: (128, 4*128);
    # block s_idx = dir*2 + side is [pi, i*64 + t], t = 32*i + 16*b2 + l
    w2s8 = np.zeros((128, 4 * 128))
    for d_ in range(2):
        w2 = dirs[d_][2]  # (16, 64)
        for side in range(2):
            blk = np.zeros((128, 2, 64))
            for i, p in enumerate(DR_PAIRS):
                sgn = _relu_sign(p, side)
                for b2 in range(2):
                    t0 = 32 * i + 16 * b2
                    blk[64 * b2 : 64 * b2 + 64, i, t0 : t0 + 16] = sgn * w2.T
            s_idx = d_ * 2 + side
            w2s8[:, s_idx * 128 : (s_idx + 1) * 128] = blk.reshape(128, 128)

    # w2m: fp16 plain stationaries for pairs {2,3}: (128, 4*32); block
    # s_idx = dir*2 + side maps (b2, j) rows -> 16*b2 + l cols
    w2m = np.zeros((128, 4 * 32))
    for d_ in range(2):
        w2 = dirs[d_][2]
        for side in range(2):
            blk = np.zeros((128, 32))
            for b2 in range(2):
                # sign may differ per pair; plain stationary is shared by
                # pairs 2 and 3, so their signs must match
                sgns = {_relu_sign(p, side) for p in (2, 3)}
                assert len(sgns) == 1, "pairs 2,3 must share side sign"
                blk[64 * b2 : 64 * b2 + 64, 16 * b2 : 16 * b2 + 16] = (
                    sgns.pop() * w2.T
                )
            s_idx = d_ * 2 + side
            w2m[:, s_idx * 32 : (s_idx + 1) * 32] = blk

    # b2t: (128, 2); output partition P = 16*b + l -> l = P % 16
    b2t = np.zeros((128, 2))
    for d_ in range(2):
        b2t[:, d_] = np.tile(dirs[d_][3], B) - B2SHIFT

    # per-core biases: (128, 64) fp32; col = q*8 + n
    in_maps = []
    for c in range(NCORES):
        biasA = np.zeros((128, 8 * NCHUNK))
        biasB = np.zeros((128, 8 * NCHUNK))
        for d_ in range(2):
            w1, b1, w2, b2 = dirs[d_]
            V = Vs[d_]
            C = V.sum(-1)  # (B, 64): full-state preact (all 16 d's)
            for p in range(NPAIR):
                q = d_ * NPAIR + p
                for n in range(NCHUNK):
                    base = (c * NCHUNK + n) * CHUNK
                    hb = (base >> np.arange(10, 16)) & 1  # bits 10..15
                    col = q * NCHUNK + n
                    for b2i in range(2):
                        b = 2 * p + b2i
                        ahi = V[b, :, 10:16] @ hb.astype(np.float64)  # (64,)
                        sl = slice(64 * b2i, 64 * b2i + 64)
                        biasA[sl, col] = ahi + b1
                        biasB[sl, col] = C[b] + b1 - ahi
        in_maps.append(
            {
                "alo": alo.astype(_f16),
                "w2s8": w2s8.astype(_f8),
                "w2m": w2m.astype(_f16),
                "b2t": b2t.astype(_f32),
                "biasA": biasA.astype(_f32),
                "biasB": biasB.astype(_f32),
            }
        )

    return in_maps, None, dirs, state


def _patch_act_tables():
    """Force every activation to resolve to natural_log_exp_and_others
    (contains Ln, Exp, Relu, Copy, Identity) so the kernel pays exactly one
    ACT table load."""
    import concourse.bacc as bacc_mod
    from concourse import hw_specs

    if getattr(bacc_mod, "_act_tables_patched", False):
        return
    orig = hw_specs.get_activation_tables

    def only_nle(arch):
        t = dict(orig(arch))
        if "natural_log_exp_and_others" in t:
            t = {
                k: (v if k == "natural_log_exp_and_others" else set())
                for k, v in t.items()
            }
        return t

    bacc_mod.get_activation_tables = only_nle
    bacc_mod._act_tables_patched = True


_NC_CACHE = {}


def build_nc(repeats=1):
    """Build and compile the 8-core SPMD Bass program (cached)."""
    if repeats in _NC_CACHE:
        return _NC_CACHE[repeats]

    import concourse.bacc as bacc
    import concourse.tile as tile
    from concourse import mybir

    _patch_act_tables()

    AF = mybir.ActivationFunctionType
    OP = mybir.AluOpType
    PM = mybir.MatmulPerfMode
    f32 = mybir.dt.float32
    f16 = mybir.dt.float16
    f8 = mybir.dt.float8e4

    nc = bacc.Bacc(
        "TRN2", target_bir_lowering=False, debug=False, num_devices=NCORES
    )

    ins = {}
    dts = {
        "alo": f16,
        "w2s8": f8,
        "w2m": f16,
        "b2t": f32,
        "biasA": f32,
        "biasB": f32,
    }
    for name, shape in (
        ("alo", (128, 8 * CHUNK)),
        ("w2s8", (128, 4 * 128)),
        ("w2m", (128, 4 * 32)),
        ("b2t", (128, 2)),
        ("biasA", (128, 8 * NCHUNK)),
        ("biasB", (128, 8 * NCHUNK)),
    ):
        ins[name] = nc.dram_tensor(name, shape, dts[name], kind="ExternalInput").ap()
    out_d = nc.dram_tensor(
        "edump", (128, 2 * NCHUNK * 2 * CHUNK), f16, kind="ExternalOutput"
    ).ap()

    with tile.TileContext(nc) as tc, ExitStack() as ctx:
        cpool = ctx.enter_context(tc.tile_pool(name="consts", bufs=1))
        xpool = ctx.enter_context(tc.tile_pool(name="xrelu", bufs=3))
        epool = ctx.enter_context(tc.tile_pool(name="eout", bufs=3))
        pp_mid = ctx.enter_context(tc.tile_pool(name="pmid", bufs=2, space="PSUM"))

        # small consts first on the sync queue (the first relus need the
        # biases); the bulky alo tensor loads in per-pair blocks on the
        # otherwise-idle Pool queue so both queues fill in parallel
        ct = {}
        for name in ins:
            shp = list(ins[name].shape)
            t = cpool.tile(shp, dts[name], tag=name, name=f"c_{name}")
            ct[name] = t
        for name in ("biasA", "biasB", "b2t", "w2s8", "w2m"):
            nc.sync.dma_start(ct[name][:], ins[name][:])
        for q in range(8):
            sl = slice(q * CHUNK, (q + 1) * CHUNK)
            nc.gpsimd.dma_start(ct["alo"][:, sl], ins["alo"][:, sl])

        rep_ctx = tc.For_i(0, repeats, 1) if repeats > 1 else None
        if rep_ctx is not None:
            rep_ctx.__enter__()

        # software pipeline: emit chunk k's head (relus + matmuls), then
        # chunk k-1's tail (exp + dma).  This keeps chunk k+1's ACT relu
        # AHEAD of exp(k) in the ACT queue, so the queue never head-of-line
        # blocks the next chunk's relus behind an exp that waits on the PE.
        chunks = [(d_, n) for d_ in range(2) for n in range(NCHUNK)]
        live = {}

        def head(k):
            d_, n = chunks[k]
            if True:
                # X8[side]: DoubleRow moving for pairs {0,1}: [128, (i, r)]
                X8 = [
                    xpool.tile([128, 2 * CHUNK], f8, tag=f"x8{s}",
                               name=f"x8{s}_{k}")
                    for s in range(2)
                ]
                # Xp[p-2][side]: plain-matmul moving for pairs {2,3} (fp8:
                # cheaper to write from DVE and costs the PE nothing extra)
                Xp = [
                    [
                        xpool.tile([128, CHUNK], f8, tag=f"xp{p}{s}",
                                   name=f"xp{p}{s}_{k}")
                        for s in range(2)
                    ]
                    for p in (2, 3)
                ]
                for p in range(NPAIR):
                    q = d_ * NPAIR + p
                    alo_blk = ct["alo"][:, q * CHUNK : (q + 1) * CHUNK]
                    col = q * NCHUNK + n
                    for side in range(2):
                        bias = (
                            ct["biasA"][:, col : col + 1]
                            if side == 0
                            else ct["biasB"][:, col : col + 1]
                        )
                        if p in DR_PAIRS:
                            i = DR_PAIRS.index(p)
                            dst = X8[side][:, i * CHUNK : (i + 1) * CHUNK]
                        else:
                            dst = Xp[p - 2][side][:]
                        eng = RELU_ENG[(p, side)]
                        if side == 0:
                            # +relu_a = max(Alo + biasA, 0)
                            if eng == "act":
                                nc.scalar.activation(
                                    dst, alo_blk, AF.Relu, bias=bias
                                )
                            else:
                                nc.vector.tensor_scalar(
                                    dst, alo_blk, bias, 0.0, OP.add, OP.max
                                )
                        else:
                            if eng == "act":
                                # +relu_b = relu(-Alo + biasB)
                                nc.scalar.activation(
                                    dst, alo_blk, AF.Relu, bias=bias,
                                    scale=-1.0
                                )
                            else:
                                # -relu_b = min(Alo - biasB, 0)
                                nc.vector.tensor_scalar(
                                    dst, alo_blk, bias, 0.0, OP.subtract,
                                    OP.min
                                )

                # second layer into one (128, 1024) PSUM tile, P = 16b + l:
                # DoubleRow (pairs 0,1) -> partitions 0:64; plain fp16
                # (pairs 2,3) -> partitions 64:128 via tile_position
                # group matmuls by PE mode (DoubleRow first, then plain) —
                # mode switches flush the PE pipeline (~190 ns each).  Each
                # matmul is split into 512-col halves: a matmul output may
                # not span a 2KB PSUM bank (s3d3_mm_num_elements).
                L2 = pp_mid.tile([128, 2 * CHUNK], f32, tag="L2", name=f"L2_{k}")
                HB = CHUNK // 512
                for side in range(2):
                    s_idx = d_ * 2 + side
                    w_blk = ct["w2s8"][
                        :, s_idx * 128 : (s_idx + 1) * 128
                    ].rearrange("p (i t) -> p i t", i=2)
                    x3 = X8[side][:].rearrange("p (i n) -> p i n", i=2)
                    for h in range(HB):
                        nc.tensor.matmul(
                            L2[0:64, side * CHUNK + h * 512 :
                               side * CHUNK + (h + 1) * 512],
                            w_blk,
                            x3[:, :, h * 512 : (h + 1) * 512],
                            start=True,
                            stop=True,
                            perf_mode=PM.DoubleRow,
                            tile_position=(0, 0),
                        )
                for side in range(2):
                    s_idx = d_ * 2 + side
                    for p in (2, 3):
                        for h in range(HB):
                            nc.tensor.matmul(
                                L2[32 * p : 32 * p + 32,
                                   side * CHUNK + h * 512 :
                                   side * CHUNK + (h + 1) * 512],
                                ct["w2m"][:, s_idx * 32 : (s_idx + 1) * 32],
                                Xp[p - 2][side][:, h * 512 : (h + 1) * 512],
                                start=True,
                                stop=True,
                                tile_position=(0, 32 * p),
                            )
                live[k] = (d_, L2)

        def tail(k):
            d_, L2 = live.pop(k)
            # E = exp(L2 + b2 - SHIFT), fp16, straight to DRAM
            E = epool.tile([128, 2 * CHUNK], f16, tag="E", name=f"E_{k}")
            nc.scalar.activation(
                E[:], L2[:], AF.Exp, bias=ct["b2t"][:, d_ : d_ + 1]
            )
            nc.sync.dma_start(
                out_d[:, k * 2 * CHUNK : (k + 1) * 2 * CHUNK], E[:]
            )

        for k in range(len(chunks)):
            head(k)
            if k > 0:
                tail(k - 1)
        tail(len(chunks) - 1)

        if rep_ctx is not None:
            rep_ctx.__exit__(None, None, None)

    nc.compile()
    _NC_CACHE[repeats] = nc
    return nc


DELTA = 0.05  # nats: candidate margin below the device max (>> fp8 noise)


def kernel(**inputs):
    from concourse.bass_utils import run_bass_kernel_spmd

    in_maps, _, dirs, state = _host_prep(inputs)
    nc = build_nc()
    res = run_bass_kernel_spmd(nc, in_maps, list(range(NCORES)))
    ed = np.stack([r["edump"].astype(np.float32) for r in res.results])

    # ed: (core, 128, 16*1024); block k=dir*8+n: [side a 512 | side b 512];
    # partition P = 16*b + l; slot value m = (c*8+n)*512 + r
    ed = ed.reshape(NCORES, B, D, 2, NCHUNK, 2, CHUNK)  # (c, b, l, dir, n, s, r)
    E = ed.transpose(3, 5, 0, 4, 6, 1, 2).reshape(2, 2, M, B, D)
    Ea, Eb = E[:, 0], E[:, 1]  # (2, M, B, D)
    Za = Ea.sum(-1, keepdims=True)
    Zb = Eb.sum(-1, keepdims=True)
    u = Ea * Zb + Eb * Za
    lnu = np.log(np.maximum(u, 1e-30))
    s_all = np.empty((2, M, B), np.float32)
    for d_ in range(2):
        w1, b1, w2, b2 = dirs[d_]

        def mlp(x):
            h = np.maximum(x @ w1.T + b1, 0.0)
            lg = h @ w2.T + b2
            lg = lg - lg.max(axis=-1, keepdims=True)
            e = np.exp(lg)
            return e / e.sum(axis=-1, keepdims=True)

        full = mlp(state)  # (B, D)
        s_all[d_] = np.einsum("mbd,bd->mb", lnu[d_], full) - np.log(
            u[d_].sum(axis=-1)
        )
    s_all[:, 0, :] = -np.inf  # slot 0 = mask 0: not a valid partition

    # exact float64 re-evaluation of near-max candidates
    out = np.zeros((2, B))
    st = state  # (B, D) float64
    for d_ in range(2):
        sm = s_all[d_]  # (M, B)
        thr = sm.max(axis=0) - DELTA
        cand = np.where((sm >= thr[None, :]).any(axis=1))[0]
        bsel = ((cand[:, None] >> np.arange(D)[None, :]) & 1).astype(
            np.float64
        )  # (K, D)
        w1, b1, w2, b2 = dirs[d_]

        def mlp(x):
            h = np.maximum(x @ w1.T + b1, 0.0)
            lg = h @ w2.T + b2
            lg = lg - lg.max(axis=-1, keepdims=True)
            e = np.exp(lg)
            return e / e.sum(axis=-1, keepdims=True)

        full = mlp(st)  # (B, D)
        sa = mlp(st[None, :, :] * bsel[:, None, :])  # (K, B, D)
        sb = mlp(st[None, :, :] * (1.0 - bsel)[:, None, :])
        mix = 0.5 * (sa + sb)
        kl = (full[None] * (np.log2(full[None]) - np.log2(mix))).sum(-1)  # (K, B)
        out[d_] = kl.min(axis=0)
    return out.astype(np.float32)


if __name__ == "__main__":
    import jax

    import reference

    cpu = jax.devices("cpu")[0]
    with jax.default_device(cpu):
        inp = reference.setup_inputs()
        inp = {k: np.asarray(jax.device_put(v, cpu)) for k, v in inp.items()}
    out = kernel(**inp)
    print(out)
